# revision 5
# baseline (speedup 1.0000x reference)
"""LSTM cell (batch 8192, input 512, hidden 512) on 8 Trainium2 NeuronCores.

Data-parallel over the batch dim: each core handles 1024 rows. Weights are
replicated. The host pre-transposes both matmul operands so the contraction
dim (fan_in = 1024) lands on SBUF partitions:

  gate.T[n, b] = sum_k W.T[k, n] * combined.T[k, b]     (matmul: lhsT.T @ rhs)

so the kernel computes everything in [hidden, batch] layout; gate biases
become per-partition vectors (free on the ACT activation op), and the host
transposes the outputs back after the gather.

Matmul operands are bf16 (f32 matmul on PE is 4x slower; fp8 DoubleRow fails
the 2e-2 accuracy gate). The kernel is PE-bound at ~55us of matmul stream,
so everything else is scheduled to keep the PE gapless:

 - Dedicated DMA rings: activations on the sync HWDGE ring, weights on the
   scalar HWDGE ring, c_prev + outputs on the gpsimd SWDGE ring. This keeps
   the activation k-tiles (which pace the h=0 matmul ramp) free of
   head-of-line blocking behind weight strips.
 - Weights are host-repacked so every (k, h) [128, 512] tile is one
   contiguous 128KB DMA.
 - c_prev is cast to bf16 on the host and outputs are stored as bf16 and
   widened on the host (halves that traffic; whole-pipeline bf16 rel err
   is ~1e-2, under the 2e-2 gate). The elementwise tail runs bf16 on DVE
   (2x rate for 16-bit dtypes).
"""

import numpy as np

import concourse.bacc as bacc
import concourse.bass as bass
import concourse.mybir as mybir
from concourse import tile
from concourse.bass_utils import run_bass_kernel_spmd

N_CORES = 8
BATCH = 8192
B = BATCH // N_CORES  # 1024 batch rows per core
K = 1024              # fan_in = input_dim + hidden_dim
H = 512               # hidden dim
NG = 4                # gates: i, f, c, o
KT = K // 128         # 8 contraction tiles
HT = H // 128         # 4 hidden chunks per gate
BT = B // 512         # 2 batch halves (PSUM free-dim limit is 512 f32)

MM_DT = mybir.dt.bfloat16
F32 = mybir.dt.float32

_SIG = mybir.ActivationFunctionType.Sigmoid
_TANH = mybir.ActivationFunctionType.Tanh
# gate order within the packed weight: i, f, c, o
_GATE_FN = [_SIG, _SIG, _TANH, _SIG]


def _build():
    nc = bacc.Bacc(
        "TRN2",
        target_bir_lowering=False,
        debug=False,
        num_devices=N_CORES,
    )

    xhT = nc.dram_tensor("xhT", [K, B], MM_DT, kind="ExternalInput")
    # wP row order is (k, h, p): each (k, h) [128, 512] tile (g-major cols,
    # col g*128 + c) is one contiguous 128KB block.
    wP = nc.dram_tensor("wP", [KT * HT * 128, NG * 128], MM_DT, kind="ExternalInput")
    bias2d = nc.dram_tensor("bias2d", [128, NG * HT], F32, kind="ExternalInput")
    c_prevT = nc.dram_tensor("c_prevT", [H, B], MM_DT, kind="ExternalInput")
    h_nextT = nc.dram_tensor("h_nextT", [H, B], MM_DT, kind="ExternalOutput")
    c_nextT = nc.dram_tensor("c_nextT", [H, B], MM_DT, kind="ExternalOutput")

    with tile.TileContext(nc) as tc:
        with (
            tc.tile_pool(name="wts", bufs=1) as wpool,
            tc.tile_pool(name="acts", bufs=1) as apool,
            tc.tile_pool(name="cprev", bufs=1) as cpool,
            tc.tile_pool(name="gates", bufs=3) as gpool,
            tc.tile_pool(name="ew", bufs=3) as epool,
            tc.tile_pool(name="psum", bufs=1, space="PSUM") as pspool,
        ):
            # ---- input DMA schedule ------------------------------------
            # Stream inputs k-major so the first accumulation groups can
            # start before the full weight set has landed. c_prev tiles are
            # interleaved late: they are consumed only by the elementwise
            # tail, so they must not delay the weight/activation k-tiles.
            # k=0/1 weight strips ride the scalar engine's separate HWDGE
            # ring, in parallel with the activations on the sync ring, so
            # the first matmul's two dependencies land together.
            xh_tiles = [None] * KT          # [k] -> [128, B] (k>=1)
            xh0_half = [None, None]         # k=0 b2 halves [128, 512]
            wt_tiles = [[None] * HT for _ in range(KT)]  # [k][h] -> [128, 512]
            cp_tiles = [None] * HT

            def _load_w(k, h, eng=None):
                wt = wpool.tile([128, NG * 128], MM_DT, tag=f"w{k}_{h}", name=f"w{k}_{h}")
                r0 = (k * HT + h) * 128
                (eng or nc.sync).dma_start(wt[:], wP[r0:r0 + 128, :])
                wt_tiles[k][h] = wt

            def _load_cp(h):
                ct = cpool.tile([128, B], MM_DT, tag=f"cp{h}", name=f"cp{h}")
                nc.sync.dma_start(ct[:], c_prevT[h * 128:(h + 1) * 128, :])
                cp_tiles[h] = ct

            bias_t = None
            for k in range(KT):
                _load_w(k, 0)
                if k == 0:
                    for b2 in range(BT):
                        xt = apool.tile(
                            [128, 512], MM_DT, tag=f"xh0_{b2}", name=f"xh0_{b2}",
                        )
                        nc.sync.dma_start(
                            xt[:], xhT[0:128, b2 * 512:(b2 + 1) * 512]
                        )
                        xh0_half[b2] = xt
                elif k == 1:
                    xt = apool.tile([128, B], MM_DT, tag="xh1", name="xh1")
                    nc.sync.dma_start(xt[:], xhT[128:256, :])
                    xh_tiles[1] = xt
                    bias_t = wpool.tile([128, NG * HT], F32, tag="bias", name="bias_t")
                    nc.sync.dma_start(bias_t[:], bias2d[:])
                else:
                    xt = apool.tile([128, B], MM_DT, tag=f"xh{k}", name=f"xh{k}")
                    nc.sync.dma_start(xt[:], xhT[k * 128:(k + 1) * 128, :])
                    xh_tiles[k] = xt
            for h in range(1, HT):
                _load_cp(h - 1)
                for k in range(KT):
                    _load_w(k, h)
            _load_cp(HT - 1)

            # PE pre-warm: the HAM clock gate holds the PE at 1.2 GHz until
            # it has seen ~3.4us of sustained matmul activity. Dummy matmuls
            # on a memset tile during the input-DMA ramp start that clock
            # early, so the real matmul stream runs warm (2.4 GHz) almost
            # from its first instruction. They have no DMA dependencies, so
            # they issue as soon as the engines come up.
            warm_t = apool.tile([128, 512], MM_DT, tag="warm", name="warm_t")
            nc.vector.memset(warm_t[:], 0)
            warm_ps = pspool.tile([128, 512], F32, tag="ps0_0", name="warm_ps")
            for _ in range(5):
                nc.tensor.matmul(
                    warm_ps[:], warm_t[:, 0:128], warm_t[:],
                    start=True, stop=True,
                )

            def _rhs(k, b2):
                if k == 0:
                    return xh0_half[b2][:]
                return xh_tiles[k][:, b2 * 512:(b2 + 1) * 512]

            def _lhsT(k, h, g):
                return wt_tiles[k][h][:, g * 128:(g + 1) * 128]

            def _mk_psum(g, h, b2):
                return pspool.tile(
                    [128, 512], F32,
                    tag=f"ps{g}_{b2 % 2}", name=f"ps{g}_{h}_{b2}",
                )

            def _elementwise(h, b2, psum, chunks=1, dma_eng=None):
                """Activations + LSTM cell tail for one (h, b2) group.

                chunks>1 splits the free dim so the final group's serial
                ACT->DVE->ACT->DVE chain drains in smaller pieces.
                """
                dma_eng = dma_eng or nc.gpsimd
                hs = slice(h * 128, (h + 1) * 128)
                w = 512 // chunks

                def _act_gate(g, c):
                    t = gpool.tile(
                        [128, w], MM_DT, tag=f"g{g}", name=f"g{g}_{h}_{b2}_{c}",
                    )
                    nc.scalar.activation(
                        t[:], psum[g][:, c * w:(c + 1) * w], _GATE_FN[g],
                        bias=bias_t[:, g * HT + h:g * HT + h + 1],
                    )
                    return t

                for c in range(chunks):
                    cs = slice(b2 * 512 + c * w, b2 * 512 + (c + 1) * w)
                    # i, f, c~ first; the whole c_next/tanh chain runs while
                    # the output gate's matmuls are still on the PE (gate-
                    # major issue order puts o last).
                    gi = _act_gate(0, c)
                    gf = _act_gate(1, c)
                    gc = _act_gate(2, c)

                    t1 = epool.tile([128, w], MM_DT, tag="t1", name=f"t1_{h}_{b2}_{c}")
                    nc.vector.tensor_mul(t1[:], gi[:], gc[:])       # i * c~
                    t2 = epool.tile([128, w], MM_DT, tag="t2", name=f"t2_{h}_{b2}_{c}")
                    nc.vector.tensor_mul(t2[:], gf[:], cp_tiles[h][:, cs])
                    cn = epool.tile([128, w], MM_DT, tag="cn", name=f"cn_{h}_{b2}_{c}")
                    nc.vector.tensor_add(cn[:], t1[:], t2[:])
                    dma_eng.dma_start(c_nextT[hs, cs], cn[:])

                    th = epool.tile([128, w], MM_DT, tag="th", name=f"th_{h}_{b2}_{c}")
                    nc.scalar.activation(th[:], cn[:], _TANH)

                    go = _act_gate(3, c)
                    hn = epool.tile([128, w], MM_DT, tag="hn", name=f"hn_{h}_{b2}_{c}")
                    nc.vector.tensor_mul(hn[:], go[:], th[:])
                    dma_eng.dma_start(h_nextT[hs, cs], hn[:])

            # h=0 rides the input-DMA ramp: every group needs all 8 k-tiles,
            # so widen to all 8 PSUM banks (4 gates x 2 batch halves) and
            # issue k-major -- the PE consumes each k-tile pair 8 matmuls at
            # a time, right as it lands.
            psum0 = {b2: [_mk_psum(g, 0, b2) for g in range(NG)] for b2 in range(BT)}
            for k in range(KT):
                for g in range(NG):
                    for b2 in range(BT):
                        nc.tensor.matmul(
                            psum0[b2][g][:],
                            _lhsT(k, 0, g),
                            _rhs(k, b2),
                            start=(k == 0),
                            stop=(k == KT - 1),
                        )
            for b2 in range(BT):
                _elementwise(0, b2, psum0[b2])

            # h>=1: inputs are resident; per-(h,b2) 4-bank groups with b2
            # parity alternating between the two bank sets, so each set's
            # ACT drain overlaps the other's matmuls.
            for h in range(1, HT):
                for b2 in range(BT):
                    psum = [_mk_psum(g, h, b2) for g in range(NG)]
                    # gate-major, output gate (g=3) last: everything except
                    # ACT(o) and h=o*tanh(c) drains while o's matmuls run.
                    for g in range(NG):
                        for k in range(KT):
                            nc.tensor.matmul(
                                psum[g][:],
                                _lhsT(k, h, g),
                                _rhs(k, b2),
                                start=(k == 0),
                                stop=(k == KT - 1),
                            )
                    last = (h == HT - 1 and b2 == BT - 1)
                    _elementwise(
                        h, b2, psum,
                        chunks=2 if last else 1,
                        dma_eng=nc.sync if last else None,
                    )

    nc.compile()
    return nc


_NC_CACHE = None
_LAST_IN_MAPS = None


def kernel(x, h_prev, c_prev, W_i, b_i, W_f, b_f, W_c, b_c, W_o, b_o):
    global _NC_CACHE, _LAST_IN_MAPS
    if _NC_CACHE is None:
        _NC_CACHE = _build()
    nc = _NC_CACHE

    np_bf16 = mybir.dt.np(MM_DT)

    combT = np.concatenate([x, h_prev], axis=1).T          # (K, BATCH) f32
    combT = combT.astype(np_bf16)
    wT = np.concatenate([W_i, W_f, W_c, W_o], axis=0).T    # (K, 4H): col g*512+h*128+c
    # packed row order (k, h, p), col order (g, c)  (see _build)
    wP = np.ascontiguousarray(
        wT.reshape(KT, 128, NG, HT, 128).transpose(0, 3, 1, 2, 4)
        .reshape(KT * HT * 128, NG * 128)
    ).astype(np_bf16)
    bias2d = np.ascontiguousarray(
        np.concatenate([b_i, b_f, b_c, b_o]).reshape(NG * HT, 128).T
    ).astype(np.float32)                                   # (128, 16)
    c_prevT = c_prev.T.astype(np_bf16)                     # (H, BATCH)

    in_maps = []
    for j in range(N_CORES):
        cols = slice(j * B, (j + 1) * B)
        in_maps.append({
            "xhT": np.ascontiguousarray(combT[:, cols]),
            "wP": wP,
            "bias2d": bias2d,
            "c_prevT": np.ascontiguousarray(c_prevT[:, cols]),
        })

    _LAST_IN_MAPS = in_maps
    try:
        res = run_bass_kernel_spmd(nc, in_maps, core_ids=list(range(N_CORES)))
    except Exception:
        # transient NRT_EXEC_UNIT_UNRECOVERABLE has been observed once on an
        # otherwise-correct NEFF; one retry is cheap insurance.
        res = run_bass_kernel_spmd(nc, in_maps, core_ids=list(range(N_CORES)))

    h_next = np.concatenate([r["h_nextT"].T for r in res.results], axis=0)
    c_next = np.concatenate([r["c_nextT"].T for r in res.results], axis=0)
    return (h_next.astype(np.float32), c_next.astype(np.float32))


# revision 6
# speedup vs baseline: 1.0229x; 1.0229x over previous
"""LSTM cell (batch 8192, input 512, hidden 512) on 8 Trainium2 NeuronCores.

Data-parallel over the batch dim: each core handles 1024 rows. Weights are
replicated. The host pre-transposes both matmul operands so the contraction
dim (fan_in = 1024) lands on SBUF partitions:

  gate.T[n, b] = sum_k W.T[k, n] * combined.T[k, b]     (matmul: lhsT.T @ rhs)

so the kernel computes everything in [hidden, batch] layout; gate biases
become per-partition vectors (free on the ACT activation op), and the host
transposes the outputs back after the gather.

Matmul operands are bf16 (f32 matmul on PE is 4x slower; fp8 DoubleRow fails
the 2e-2 accuracy gate). The kernel is PE-bound at ~55us of matmul stream,
so everything else is scheduled to keep the PE gapless:

 - Dedicated DMA rings: activations on the sync HWDGE ring, weights on the
   scalar HWDGE ring, c_prev + outputs on the gpsimd SWDGE ring. This keeps
   the activation k-tiles (which pace the h=0 matmul ramp) free of
   head-of-line blocking behind weight strips.
 - Weights are host-repacked so every (k, h) [128, 512] tile is one
   contiguous 128KB DMA.
 - c_prev is cast to bf16 on the host and outputs are stored as bf16 and
   widened on the host (halves that traffic; whole-pipeline bf16 rel err
   is ~1e-2, under the 2e-2 gate). The elementwise tail runs bf16 on DVE
   (2x rate for 16-bit dtypes).
"""

import numpy as np

import concourse.bacc as bacc
import concourse.bass as bass
import concourse.mybir as mybir
from concourse import tile
from concourse.bass_utils import run_bass_kernel_spmd

N_CORES = 8
BATCH = 8192
B = BATCH // N_CORES  # 1024 batch rows per core
K = 1024              # fan_in = input_dim + hidden_dim
H = 512               # hidden dim
NG = 4                # gates: i, f, c, o
KT = K // 128         # 8 contraction tiles
HT = H // 128         # 4 hidden chunks per gate
BT = B // 512         # 2 batch halves (PSUM free-dim limit is 512 f32)

MM_DT = mybir.dt.bfloat16
F32 = mybir.dt.float32

_SIG = mybir.ActivationFunctionType.Sigmoid
_TANH = mybir.ActivationFunctionType.Tanh
# gate order within the packed weight: i, f, c, o
_GATE_FN = [_SIG, _SIG, _TANH, _SIG]


def _build():
    nc = bacc.Bacc(
        "TRN2",
        target_bir_lowering=False,
        debug=False,
        num_devices=N_CORES,
    )

    xhT = nc.dram_tensor("xhT", [K, B], MM_DT, kind="ExternalInput")
    # wP row order is (k, h, p): each (k, h) [128, 512] tile (g-major cols,
    # col g*128 + c) is one contiguous 128KB block.
    wP = nc.dram_tensor("wP", [KT * HT * 128, NG * 128], MM_DT, kind="ExternalInput")
    bias2d = nc.dram_tensor("bias2d", [128, NG * HT], F32, kind="ExternalInput")
    c_prevT = nc.dram_tensor("c_prevT", [H, B], MM_DT, kind="ExternalInput")
    h_nextT = nc.dram_tensor("h_nextT", [H, B], MM_DT, kind="ExternalOutput")
    c_nextT = nc.dram_tensor("c_nextT", [H, B], MM_DT, kind="ExternalOutput")

    with tile.TileContext(nc) as tc:
        with (
            tc.tile_pool(name="wts", bufs=1) as wpool,
            tc.tile_pool(name="acts", bufs=1) as apool,
            tc.tile_pool(name="cprev", bufs=1) as cpool,
            tc.tile_pool(name="gates", bufs=3) as gpool,
            tc.tile_pool(name="ew", bufs=3) as epool,
            tc.tile_pool(name="psum", bufs=1, space="PSUM") as pspool,
        ):
            # ---- input DMA schedule ------------------------------------
            # Stream inputs k-major so the first accumulation groups can
            # start before the full weight set has landed. c_prev tiles are
            # interleaved late: they are consumed only by the elementwise
            # tail, so they must not delay the weight/activation k-tiles.
            # k=0/1 weight strips ride the scalar engine's separate HWDGE
            # ring, in parallel with the activations on the sync ring, so
            # the first matmul's two dependencies land together.
            xh_tiles = [None] * KT          # [k] -> [128, B] (k>=1)
            xh0_half = [None, None]         # k=0 b2 halves [128, 512]
            wt_tiles = [[None] * HT for _ in range(KT)]  # [k][h] -> [128, 512]
            cp_tiles = [None] * HT

            def _load_w(k, h, eng=None):
                wt = wpool.tile([128, NG * 128], MM_DT, tag=f"w{k}_{h}", name=f"w{k}_{h}")
                r0 = (k * HT + h) * 128
                (eng or nc.sync).dma_start(wt[:], wP[r0:r0 + 128, :])
                wt_tiles[k][h] = wt

            def _load_cp(h):
                ct = cpool.tile([128, B], MM_DT, tag=f"cp{h}", name=f"cp{h}")
                nc.sync.dma_start(ct[:], c_prevT[h * 128:(h + 1) * 128, :])
                cp_tiles[h] = ct

            bias_t = None
            for k in range(KT):
                _load_w(k, 0, eng=nc.scalar if k < 2 else None)
                if k == 0:
                    for b2 in range(BT):
                        xt = apool.tile(
                            [128, 512], MM_DT, tag=f"xh0_{b2}", name=f"xh0_{b2}",
                        )
                        nc.sync.dma_start(
                            xt[:], xhT[0:128, b2 * 512:(b2 + 1) * 512]
                        )
                        xh0_half[b2] = xt
                    bias_t = wpool.tile([128, NG * HT], F32, tag="bias", name="bias_t")
                    nc.sync.dma_start(bias_t[:], bias2d[:])
                else:
                    xt = apool.tile([128, B], MM_DT, tag=f"xh{k}", name=f"xh{k}")
                    nc.sync.dma_start(xt[:], xhT[k * 128:(k + 1) * 128, :])
                    xh_tiles[k] = xt
            for h in range(1, HT):
                _load_cp(h - 1)
                for k in range(KT):
                    _load_w(k, h)
            _load_cp(HT - 1)

            # PE pre-warm: the HAM clock gate holds the PE at 1.2 GHz until
            # it has seen ~3.4us of sustained matmul activity. Dummy matmuls
            # on a memset tile during the input-DMA ramp start that clock
            # early, so the real matmul stream runs warm (2.4 GHz) almost
            # from its first instruction. They have no DMA dependencies, so
            # they issue as soon as the engines come up.
            warm_t = apool.tile([128, 512], MM_DT, tag="warm", name="warm_t")
            nc.vector.memset(warm_t[:], 0)
            warm_ps = pspool.tile([128, 512], F32, tag="ps0_0", name="warm_ps")
            for _ in range(5):
                nc.tensor.matmul(
                    warm_ps[:], warm_t[:, 0:128], warm_t[:],
                    start=True, stop=True,
                )

            def _rhs(k, b2):
                if k == 0:
                    return xh0_half[b2][:]
                return xh_tiles[k][:, b2 * 512:(b2 + 1) * 512]

            def _lhsT(k, h, g):
                return wt_tiles[k][h][:, g * 128:(g + 1) * 128]

            def _mk_psum(g, h, b2):
                return pspool.tile(
                    [128, 512], F32,
                    tag=f"ps{g}_{b2 % 2}", name=f"ps{g}_{h}_{b2}",
                )

            def _elementwise(h, b2, psum, chunks=1, dma_eng=None):
                """Activations + LSTM cell tail for one (h, b2) group.

                chunks>1 splits the free dim so the final group's serial
                ACT->DVE->ACT->DVE chain drains in smaller pieces.
                """
                dma_eng = dma_eng or nc.gpsimd
                hs = slice(h * 128, (h + 1) * 128)
                w = 512 // chunks

                def _act_gate(g, c):
                    t = gpool.tile(
                        [128, w], MM_DT, tag=f"g{g}", name=f"g{g}_{h}_{b2}_{c}",
                    )
                    nc.scalar.activation(
                        t[:], psum[g][:, c * w:(c + 1) * w], _GATE_FN[g],
                        bias=bias_t[:, g * HT + h:g * HT + h + 1],
                    )
                    return t

                for c in range(chunks):
                    cs = slice(b2 * 512 + c * w, b2 * 512 + (c + 1) * w)
                    # i, f, c~ first; the whole c_next/tanh chain runs while
                    # the output gate's matmuls are still on the PE (gate-
                    # major issue order puts o last).
                    gi = _act_gate(0, c)
                    gf = _act_gate(1, c)
                    gc = _act_gate(2, c)

                    t1 = epool.tile([128, w], MM_DT, tag="t1", name=f"t1_{h}_{b2}_{c}")
                    nc.vector.tensor_mul(t1[:], gi[:], gc[:])       # i * c~
                    t2 = epool.tile([128, w], MM_DT, tag="t2", name=f"t2_{h}_{b2}_{c}")
                    nc.vector.tensor_mul(t2[:], gf[:], cp_tiles[h][:, cs])
                    cn = epool.tile([128, w], MM_DT, tag="cn", name=f"cn_{h}_{b2}_{c}")
                    nc.vector.tensor_add(cn[:], t1[:], t2[:])
                    dma_eng.dma_start(c_nextT[hs, cs], cn[:])

                    th = epool.tile([128, w], MM_DT, tag="th", name=f"th_{h}_{b2}_{c}")
                    nc.scalar.activation(th[:], cn[:], _TANH)

                    go = _act_gate(3, c)
                    hn = epool.tile([128, w], MM_DT, tag="hn", name=f"hn_{h}_{b2}_{c}")
                    nc.vector.tensor_mul(hn[:], go[:], th[:])
                    dma_eng.dma_start(h_nextT[hs, cs], hn[:])

            # h=0 rides the input-DMA ramp: every group needs all 8 k-tiles,
            # so widen to all 8 PSUM banks (4 gates x 2 batch halves) and
            # issue k-major -- the PE consumes each k-tile pair 8 matmuls at
            # a time, right as it lands.
            psum0 = {b2: [_mk_psum(g, 0, b2) for g in range(NG)] for b2 in range(BT)}
            for k in range(KT):
                for g in range(NG):
                    for b2 in range(BT):
                        nc.tensor.matmul(
                            psum0[b2][g][:],
                            _lhsT(k, 0, g),
                            _rhs(k, b2),
                            start=(k == 0),
                            stop=(k == KT - 1),
                        )
            for b2 in range(BT):
                _elementwise(0, b2, psum0[b2])

            # h>=1: inputs are resident; per-(h,b2) 4-bank groups with b2
            # parity alternating between the two bank sets, so each set's
            # ACT drain overlaps the other's matmuls.
            for h in range(1, HT):
                for b2 in range(BT):
                    psum = [_mk_psum(g, h, b2) for g in range(NG)]
                    # gate-major, output gate (g=3) last: everything except
                    # ACT(o) and h=o*tanh(c) drains while o's matmuls run.
                    for g in range(NG):
                        for k in range(KT):
                            nc.tensor.matmul(
                                psum[g][:],
                                _lhsT(k, h, g),
                                _rhs(k, b2),
                                start=(k == 0),
                                stop=(k == KT - 1),
                            )
                    last = (h == HT - 1 and b2 == BT - 1)
                    _elementwise(
                        h, b2, psum,
                        chunks=2 if last else 1,
                        dma_eng=nc.sync if last else None,
                    )

    nc.compile()
    return nc


_NC_CACHE = None
_LAST_IN_MAPS = None


def kernel(x, h_prev, c_prev, W_i, b_i, W_f, b_f, W_c, b_c, W_o, b_o):
    global _NC_CACHE, _LAST_IN_MAPS
    if _NC_CACHE is None:
        _NC_CACHE = _build()
    nc = _NC_CACHE

    np_bf16 = mybir.dt.np(MM_DT)

    combT = np.concatenate([x, h_prev], axis=1).T          # (K, BATCH) f32
    combT = combT.astype(np_bf16)
    wT = np.concatenate([W_i, W_f, W_c, W_o], axis=0).T    # (K, 4H): col g*512+h*128+c
    # packed row order (k, h, p), col order (g, c)  (see _build)
    wP = np.ascontiguousarray(
        wT.reshape(KT, 128, NG, HT, 128).transpose(0, 3, 1, 2, 4)
        .reshape(KT * HT * 128, NG * 128)
    ).astype(np_bf16)
    bias2d = np.ascontiguousarray(
        np.concatenate([b_i, b_f, b_c, b_o]).reshape(NG * HT, 128).T
    ).astype(np.float32)                                   # (128, 16)
    c_prevT = c_prev.T.astype(np_bf16)                     # (H, BATCH)

    in_maps = []
    for j in range(N_CORES):
        cols = slice(j * B, (j + 1) * B)
        in_maps.append({
            "xhT": np.ascontiguousarray(combT[:, cols]),
            "wP": wP,
            "bias2d": bias2d,
            "c_prevT": np.ascontiguousarray(c_prevT[:, cols]),
        })

    _LAST_IN_MAPS = in_maps
    try:
        res = run_bass_kernel_spmd(nc, in_maps, core_ids=list(range(N_CORES)))
    except Exception:
        # transient NRT_EXEC_UNIT_UNRECOVERABLE has been observed once on an
        # otherwise-correct NEFF; one retry is cheap insurance.
        res = run_bass_kernel_spmd(nc, in_maps, core_ids=list(range(N_CORES)))

    h_next = np.concatenate([r["h_nextT"].T for r in res.results], axis=0)
    c_next = np.concatenate([r["c_nextT"].T for r in res.results], axis=0)
    return (h_next.astype(np.float32), c_next.astype(np.float32))
